# revision 30
# baseline (speedup 1.0000x reference)
"""Trainium2 Bass kernel for nn_Downsampler: depthwise 9x9 conv, stride 4,
pad 4 over input (4, 64, 512, 512) fp32 with a single shared [1,1,9,9] kernel.

Strategy
--------
The 256 independent (batch, channel) images are sharded 32-per-core across
8 NeuronCores (data parallel). Per image, the conv is computed entirely on
the tensor engine via a polyphase banded-matmul decomposition:

  * A 512x512 image reshaped to SBUF [128, 2048] puts rows {4p..4p+3} on
    partition p -- free index = rho*512 + x after a (p r) c -> p (r c)
    regroup, i.e. the row-phase split falls out of a plain reshape.
  * Writing dy-4 = 4a+rho, dx-4 = 4b+sigma (a,b in {-1,0,1}), the conv
    splits into 36 matmuls (4 rho x 9 (sigma,b) pairs), each contracting
    over the 128 partitions with a banded stationary matrix
    W[p, yo] = K[4(p-yo)+rho+4, 4b+sigma+4] (diagonals a = p-yo), and
    streaming rhs = tile[:, rho*512 + sigma + 4b + 4*xo] (stride-4 free
    dim). All 36 accumulate natively into one PSUM tile [128 yo, xo].
  * Row padding is implicit in the band clipping to partitions [0,128);
    column padding is handled by clipping the xo range for b = +/-1.
  * 4 images share each matmul's moving operand (free dim = 4*128 = 512
    columns, one PSUM bank), so the stationary load (107 ns) hides under
    the 213 ns stream and the PE runs at its streaming roofline.

Inputs are cast fp32->bf16 during the DMA (SWDGE); the banded weights are
built host-side from the 9x9 kernel input and loaded once per core. bf16
keeps the stride-4 rhs reads at 2 hits per 16B SBUF line (full rate) and
gives ~2e-3 relative error, far inside tolerance. Input tiles are loaded
with one sub-DMA per row-phase rho, so the tensor engine starts as soon as
the first quarter lands and streams with no idle gaps (keeping the PE HAM
clock-gate warm); the kernel is HBM-read bound at roofline.
"""

import numpy as np
import ml_dtypes

import concourse.bass as bass
import concourse.mybir as mybir
from concourse.tile import TileContext
from concourse.bass_utils import run_bass_kernel_spmd

N_CORES = 8
B, C, H, W = 4, 64, 512, 512
KS = 9
HO = WO = 128
IMGS = B * C                    # 256 independent images
PER_CORE = IMGS // N_CORES      # 32
GROUP = 4                       # images per PSUM accumulation group
N_GROUPS = PER_CORE // GROUP    # 8
N_W = 36                        # 4 rho x 9 (sigma, b)

# (sigma, b) pairs covering dx = 4b + sigma + 4 in [0, 9). b=0 pairs first so
# the start=True matmul covers every PSUM column of the accumulation group.
SB_PAIRS = [(0, 0), (1, 0), (2, 0), (3, 0),
            (0, -1), (1, -1), (2, -1), (3, -1), (0, 1)]


def build_weights(kernel2d: np.ndarray, as_f32: bool = False) -> np.ndarray:
    """[9,9] fp32 -> [128, 36*128] bf16 stationary matrices, laid out
    wt[p, widx*128 + yo] with widx = rho*9 + j over SB_PAIRS order."""
    Ws = np.zeros((4, len(SB_PAIRS), 128, 128), np.float32)  # [rho, j, p, yo]
    for rho in range(4):
        for j, (sigma, b) in enumerate(SB_PAIRS):
            dx = 4 * b + sigma + 4
            for a in (-1, 0, 1):
                dy = 4 * a + rho + 4
                if 0 <= dy < KS:
                    yos = np.arange(max(0, -a), min(128, 128 - a))
                    Ws[rho, j, yos + a, yos] = kernel2d[dy, dx]
    wt = Ws.reshape(N_W, 128, 128).transpose(1, 0, 2).reshape(128, N_W * 128)
    return np.ascontiguousarray(
        wt if as_f32 else wt.astype(ml_dtypes.bfloat16))


_PROG = None


def _split_multi_waits(nc: bass.Bass) -> None:
    """This walrus build accepts at most ONE sem wait per instruction (the
    TPB_EVENTS field has a single wait slot), but Tile attaches 2+ waits to
    instructions whose operand tiles were last touched by several different
    processors. Rewrite: keep one wait on the instruction and move every
    extra wait onto its own nop on the same engine, placed immediately
    before it -- engine sequencers are in-order, so gating a preceding nop
    gates the instruction.
    """
    for f in nc.m.functions:
        for blk in f.blocks:
            insts = blk.instructions
            patched = []
            for inst in insts:
                si = inst.sync_info
                if si is not None and si.on_wait and len(si.on_wait) > 1:
                    for wait in si.on_wait[:-1]:
                        nop = nc.engines[inst.engine].nop(
                            hint="wait_split").ins
                        # engine nop() appended itself somewhere; pull it out
                        for b2 in f.blocks:
                            if b2.instructions and b2.instructions[-1] is nop:
                                b2.instructions.pop()
                                break
                        nop.sync_info = mybir.SyncInfo(on_wait=[wait],
                                                       on_update=[])
                        patched.append(nop)
                    inst.sync_info = mybir.SyncInfo(
                        on_wait=[si.on_wait[-1]],
                        on_update=list(si.on_update))
                patched.append(inst)
            blk.instructions[:] = patched


def _build_program(repeats: int = 1, dma_groups: int = 1, sw_cast: bool = False,
                   xf_bufs: int = 3, xt_bufs: int = 2, dma_split: int = 1,
                   out_sync: bool = False, ps_bufs: int = 2,
                   rho_split: bool = False,
                   timing_stub: bool = False) -> bass.Bass:
    """dma_groups: number of 4-image GROUPs fetched per input DMA tile.
    dma_split: split each input tile's DMA into this many sub-DMAs (along
    the image dim) writing disjoint slices of the same tile.
    sw_cast: cast fp32->bf16 inside the (SWDGE) input DMA instead of
    staging fp32 and casting on the vector engine.
    timing_stub: x/y live in internal DRAM (garbage data, same compute) so
    per-call host<->device transfers are tiny; for timing only."""
    nc = bass.Bass()
    if timing_stub:
        x = nc.dram_tensor("x_int", [PER_CORE, H, W], mybir.dt.float32)[:]
        y = nc.dram_tensor("y_int", [PER_CORE, HO, WO], mybir.dt.float32)[:]
        nc.declare_dram_parameter("tok", [1, 1], mybir.dt.float32,
                                  isOutput=True)
    else:
        x = nc.declare_dram_parameter("x", [PER_CORE, H, W], mybir.dt.float32,
                                      isOutput=False)
        y = nc.declare_dram_parameter("y", [PER_CORE, HO, WO],
                                      mybir.dt.float32, isOutput=True)
    w = nc.declare_dram_parameter("w", [128, N_W * 128], mybir.dt.bfloat16,
                                  isOutput=False)
    DG = dma_groups * GROUP  # images per input DMA

    with TileContext(nc) as tc:
        with tc.tile_pool(name="wp", bufs=1) as wp, \
             tc.tile_pool(name="xf", bufs=xf_bufs) as xfp, \
             tc.tile_pool(name="xp", bufs=xt_bufs) as xp, \
             tc.tile_pool(name="op", bufs=3) as op, \
             tc.tile_pool(name="pp", bufs=ps_bufs, space="PSUM") as pp:
            wt = wp.tile([128, N_W * 128], mybir.dt.bfloat16)
            nc.sync.dma_start(out=wt[:], in_=w[:])

            state = {"xf": None}

            def emit_group(gi):
                if gi % dma_groups == 0:
                    # Input load. Layout [p, g, rho*512+x]: partition p holds
                    # image rows 4p..4p+3 (a plain reshape of the image).
                    dt_in = (mybir.dt.bfloat16 if sw_cast
                             else mybir.dt.float32)
                    dma = nc.gpsimd if sw_cast else nc.sync
                    xf = xfp.tile([128, DG * 2048], dt_in, tag="xf")
                    xfv = xf[:].rearrange("p (g c) -> p g c", g=DG)
                    if rho_split:
                        # One sub-DMA per row-phase rho: each unlocks the 9
                        # matmuls of that rho, so the PE never idles past the
                        # HAM MID window between groups.
                        xfv4 = xf[:].rearrange("p (g r c) -> p g r c",
                                               g=DG, r=4)
                        src4 = x[gi * GROUP:gi * GROUP + DG].rearrange(
                            "g (p r) c -> p g r c", r=4)
                        for s in range(4):
                            dma.dma_start(out=xfv4[:, :, s, :],
                                          in_=src4[:, :, s, :])
                    else:
                        sub_n = DG // dma_split
                        for s in range(dma_split):
                            src = x[gi * GROUP + s * sub_n:
                                    gi * GROUP + (s + 1) * sub_n].rearrange(
                                "g (p r) c -> p g (r c)", r=4)
                            dma.dma_start(
                                out=xfv[:, s * sub_n:(s + 1) * sub_n],
                                in_=src)
                    state["xf"] = xf
                xf = state["xf"]
                sub = gi % dma_groups
                xfg = xf[:, sub * GROUP * 2048:(sub + 1) * GROUP * 2048]
                if sw_cast:
                    xv = xfg.rearrange("p (g c) -> p g c", g=GROUP)
                else:
                    xt = xp.tile([128, GROUP * 2048], mybir.dt.bfloat16,
                                 tag="xt")
                    nc.vector.tensor_copy(xt[:], xfg)
                    xv = xt[:].rearrange("p (g c) -> p g c", g=GROUP)

                ps = pp.tile([128, GROUP * WO], mybir.dt.float32, tag="ps")
                pv = ps[:].rearrange("p (g m) -> p g m", g=GROUP)

                k = 0
                for rho in range(4):
                    for j, (sigma, b) in enumerate(SB_PAIRS):
                        widx = rho * len(SB_PAIRS) + j
                        lo = 1 if b == -1 else 0
                        hi = 127 if b == 1 else 128
                        cnt = hi - lo
                        off = rho * 512 + 4 * lo + 4 * b + sigma
                        rhs = xv[:, :, off:off + 4 * (cnt - 1) + 1:4]
                        out = pv[:, :, lo:hi]
                        nc.tensor.matmul(
                            out, wt[:, widx * 128:(widx + 1) * 128],
                            rhs, start=(k == 0), stop=(k == N_W - 1))
                        k += 1

                ot = op.tile([128, GROUP * WO], mybir.dt.float32, tag="ot")
                nc.scalar.copy(ot[:], ps[:])
                dst = y[gi * GROUP:(gi + 1) * GROUP].rearrange(
                    "g yo xo -> yo g xo")
                out_eng = nc.sync if out_sync else nc.scalar
                out_eng.dma_start(
                    out=dst, in_=ot[:].rearrange("p (g m) -> p g m", g=GROUP))

            for gi in [g for _ in range(repeats)
                       for g in range(N_GROUPS)]:
                emit_group(gi)

    _split_multi_waits(nc)
    return nc


# Tuned configuration (measured ~95-107us/core steady-state vs the ~95us
# HBM-read roofline): SWDGE cast DMA over 8-image tiles, split into one
# sub-DMA per row-phase rho so each 2MB transfer unlocks that rho's matmuls
# -- the PE streams gap-free (cost-model sim: zero PE gaps >0.5us) and never
# re-throttles through the HAM MID window.
BEST_CFG = dict(dma_groups=2, sw_cast=True, xf_bufs=3, xt_bufs=1,
                rho_split=True)


def _get_program() -> bass.Bass:
    global _PROG
    if _PROG is None:
        _PROG = _build_program(**BEST_CFG)
    return _PROG


def run(input0, kernel, trace=False, **spmd_kwargs):
    """Shard, run on 8 cores, gather. Returns (output, BassKernelResults)."""
    x = np.ascontiguousarray(
        np.asarray(input0, dtype=np.float32).reshape(IMGS, H, W))
    k2 = np.asarray(kernel, dtype=np.float32).reshape(KS, KS)
    wt = build_weights(k2)
    nc = _get_program()
    in_maps = [
        {"x": x[i * PER_CORE:(i + 1) * PER_CORE], "w": wt}
        for i in range(N_CORES)
    ]
    res = run_bass_kernel_spmd(nc, in_maps, list(range(N_CORES)),
                               trace=trace, **spmd_kwargs)
    out = np.concatenate([np.asarray(res.results[i]["y"])
                          for i in range(N_CORES)], axis=0)
    return out.reshape(B, C, HO, WO).astype(np.float32, copy=False), res


def kernel(**inputs) -> np.ndarray:
    out, _ = run(inputs["input0"], inputs["kernel"])
    return out


class Runner:
    """Cached jitted executable over 8 cores with device-resident inputs,
    for wall-clock timing without per-call retrace/transfer overhead."""

    def __init__(self, nc=None):
        import jax
        from jax.sharding import Mesh, PartitionSpec
        from jax.experimental.shard_map import shard_map
        from concourse import bass2jax

        bass2jax.install_neuronx_cc_hook()
        nc = nc or _get_program()
        self.nc = nc
        pid_name = (nc.partition_id_tensor.name
                    if nc.partition_id_tensor else None)
        in_names, out_names, out_avals, zero_outs = [], [], [], []
        for alloc in nc.m.functions[0].allocations:
            if not isinstance(alloc, mybir.MemoryLocationSet):
                continue
            name = alloc.memorylocations[0].name
            if alloc.kind == "ExternalInput":
                if name != pid_name:
                    in_names.append(name)
            elif alloc.kind == "ExternalOutput":
                out_names.append(name)
                shape = tuple(alloc.tensor_shape)
                dtype = mybir.dt.np(alloc.dtype)
                out_avals.append(jax.core.ShapedArray(shape, dtype))
                zero_outs.append(np.zeros(shape, dtype))
        self.in_names, self.out_names = in_names, out_names
        self.zero_outs = zero_outs

        bind_names = list(in_names) + list(out_names)
        if pid_name is not None:
            bind_names.append(pid_name)

        def _body(*args):
            operands = list(args)
            if pid_name is not None:
                operands.append(bass2jax.partition_id_tensor())
            return tuple(bass2jax._bass_exec_p.bind(
                *operands,
                out_avals=tuple(out_avals),
                in_names=tuple(bind_names),
                out_names=tuple(out_names),
                lowering_input_output_aliases=(),
                sim_require_finite=True,
                sim_require_nnan=True,
                nc=nc,
            ))

        devices = jax.devices()[:N_CORES]
        mesh = Mesh(np.asarray(devices), ("core",))
        nargs = len(in_names) + len(out_names)
        self._fn = jax.jit(
            shard_map(_body, mesh=mesh,
                      in_specs=(PartitionSpec("core"),) * nargs,
                      out_specs=(PartitionSpec("core"),) * len(out_names),
                      check_rep=False),
            keep_unused=True)
        self._jax = jax

    def put(self, in_maps):
        jax = self._jax
        args = []
        for name in self.in_names:
            args.append(np.concatenate(
                [np.asarray(m[name]) for m in in_maps], axis=0))
        for z in self.zero_outs:
            args.append(np.concatenate([z] * N_CORES, axis=0))
        return [jax.device_put(a) for a in args]

    def __call__(self, args):
        outs = self._fn(*args)
        self._jax.block_until_ready(outs)
        return outs


def _build_null_program() -> bass.Bass:
    """Minimal kernel (tiny copy) to measure per-call dispatch overhead."""
    nc = bass.Bass()
    x = nc.declare_dram_parameter("x", [128, 128], mybir.dt.float32,
                                  isOutput=False)
    y = nc.declare_dram_parameter("y", [128, 128], mybir.dt.float32,
                                  isOutput=True)
    with TileContext(nc) as tc:
        with tc.tile_pool(name="t", bufs=1) as tp:
            t = tp.tile([128, 128], mybir.dt.float32)
            nc.sync.dma_start(out=t[:], in_=x[:])
            nc.sync.dma_start(out=y[:], in_=t[:])
    _split_multi_waits(nc)
    return nc
